# revision 4
# baseline (speedup 1.0000x reference)
"""Trainium2 Bass kernel for LoRA-segmented linear layer.

Computes y = x @ W^T + bias + scalings[e] * (x_e @ A_e^T) @ B_e^T
where x is split into 8 equal contiguous token segments (one per adapter).

Sharding: data-parallel over tokens; core e gets segment e (4096 tokens),
which exactly matches adapter e, so all LoRA work is core-local.

Per-core device kernel:
  0. Warmup matmuls on a zeroed tile keep the PE busy from t=0 so the HAM
     clock gate reaches 8/8 (2.4 GHz) before real work arrives, and stays
     there through the DMA-bound startup phase.
  1. Fold LoRA into an effective weight on-device:
       W_eff^T = W^T + A_e^T @ (s_e * B_e^T)
     The rank-16 product goes to PSUM on the PE; evacuation to SBUF is
     split across two engines to keep it off the critical path:
       - d_out columns 0-1023: DVE tensor_add(psum, W_tile)
       - d_out columns 1024-2047: an identity matmul accumulates the W tile
         into PSUM on the (idle) PE, then the scalar/ACT engine copies
         psum -> sbuf (bf16 downcast). ACT cannot add tensors, but it can
         copy, and the PE has slack during the DMA-bound start.
  2. Dense GEMM y_e = x_e @ W_eff^T + bias, tiled as:
       stationary = x^T tile [128(d) x 128(tok)], moving = W_eff^T [128 x 512]
       PSUM accumulates fp32 over the 16 k-tiles; DVE adds bias; one
       1 MB DMA per 128-token row block writes y out on the scalar ring.

DMAs are batched (one per x chunk / W wave / output row-block) to cut
trigger overhead, with W and x on the sync engine's HWDGE ring ordered
x0 -> W -> x1.. so startup transfers have strict priority, and y on the
scalar engine's ring.

After TileContext exit we deduplicate InstLdweights: the tile legalizer
emits one LDWEIGHTS per matmul, but consecutive matmuls that share the
same stationary AP (the 4 d_out chunks per x-tile) only need the first.
This cuts the PE issue gap from ~259ns/MM to the ~216ns streaming floor.

Host-side prep: transpose x/W, cast to bf16, pre-scale B by scalings.
"""

import numpy as np
import ml_dtypes

# Problem geometry (hardcoded per contest contract).
N_TOK, D_IN, D_OUT, E, R = 32768, 2048, 2048, 8, 16
S = N_TOK // E          # tokens per core / segment: 4096
P = 128                 # partitions
NK = D_IN // P          # 16 contraction tiles
TCH = 512               # token chunk (x dma width per k-tile)
NCH = S // TCH          # 8 token chunks per core
M_PER = TCH // P        # 4 m-subtiles (of 128 tokens) per chunk
OC = 512                # dout chunk (matmul moving free dim; one PSUM bank)
NOC = D_OUT // OC       # 4 dout chunks
WAVE = 4                # k-tiles per W DMA wave
NWAVE = NK // WAVE      # 4 waves
N_WARM = 40             # warmup matmuls bridge t=0 to the first W arrival

_PROGRAM = None         # cached Bass program
LAST_RESULTS = None     # BassKernelResults of the most recent run (for profiling)


def _dedup_ldweights(nc):
    """Remove InstLdweights that reload the stationary AP already resident
    (identical AP string, only matmults in between). Any waits on a removed
    LDW are moved onto the next matmult. Must run after TileContext exit and
    before nc.finalize()."""
    import concourse.mybir as mybir

    n_removed = 0
    for fn in nc.m.functions:
        for bb in fn.blocks:
            insts = list(bb.instructions)
            keep = []
            last_key = None
            pending_waits = []
            removed_here = False
            for i in insts:
                tn = type(i).__name__
                if tn == "InstLdweights":
                    key = str(i.ins[0])
                    if key == last_key:
                        si = i.sync_info
                        if si is not None and si.on_wait:
                            pending_waits.extend(si.on_wait)
                        n_removed += 1
                        removed_here = True
                        continue
                    last_key = key
                elif tn == "InstMatmult":
                    if pending_waits:
                        si = i.sync_info
                        if si is None:
                            i.sync_info = mybir.SyncInfo(
                                on_wait=list(pending_waits), on_update=[])
                        else:
                            si.on_wait = list(si.on_wait) + pending_waits
                            i.sync_info = si
                        pending_waits = []
                else:
                    last_key = None
                keep.append(i)
            assert not pending_waits
            if removed_here:
                bb.instructions = keep
    return n_removed


def _build_program(in_dt_name="bfloat16"):
    from contextlib import ExitStack

    import concourse.mybir as mybir
    import concourse.tile as tile
    from concourse import bacc

    in_dt = getattr(mybir.dt, in_dt_name)
    f32 = mybir.dt.float32
    COPY = mybir.ActivationFunctionType.Copy

    nc = bacc.Bacc(trn_type="TRN2")

    xt = nc.dram_tensor("xt", [D_IN, S], in_dt, kind="ExternalInput")
    wt = nc.dram_tensor("wt", [D_IN, D_OUT], in_dt, kind="ExternalInput")
    bias_d = nc.dram_tensor("bias", [D_OUT], f32, kind="ExternalInput")
    at = nc.dram_tensor("at", [R, D_IN], in_dt, kind="ExternalInput")
    sbt = nc.dram_tensor("sbt", [R, D_OUT], in_dt, kind="ExternalInput")
    ident_d = nc.dram_tensor("ident", [P, P], in_dt, kind="ExternalInput")
    y = nc.dram_tensor("y", [S, D_OUT], f32, kind="ExternalOutput")

    with ExitStack() as ctx:
        tc = ctx.enter_context(tile.TileContext(nc))
        persist = ctx.enter_context(tc.tile_pool(name="persist", bufs=1))
        wpool = ctx.enter_context(tc.tile_pool(name="wpool", bufs=2))
        xp = ctx.enter_context(tc.tile_pool(name="xp", bufs=3))
        outp = ctx.enter_context(tc.tile_pool(name="outp", bufs=3))
        psum = ctx.enter_context(tc.tile_pool(name="psum", bufs=8, space="PSUM"))

        # --- HAM warmup: PE busy from t=0 so the clock ungates early ---
        warm = persist.tile([P, OC], in_dt, tag="warm", name="warm_sb")
        nc.vector.memset(warm, 0.0)
        wps = psum.tile([P, OC], f32, tag="ps", name="warm_ps")
        for i in range(N_WARM):
            nc.tensor.matmul(wps, warm[:, :P], warm[:], start=True, stop=True)

        # --- persistent small tensors ---
        bias_sb = persist.tile([P, D_OUT], f32, tag="bias", name="bias_sb")
        # stride-0 partition broadcast must go via SW DGE (gpsimd), not HW DGE
        nc.gpsimd.dma_start(out=bias_sb, in_=bias_d[:].partition_broadcast(P))
        at_sb = persist.tile([R, D_IN], in_dt, tag="at", name="at_sb")
        nc.sync.dma_start(out=at_sb, in_=at[:])
        sbt_sb = persist.tile([R, D_OUT], in_dt, tag="sbt", name="sbt_sb")
        nc.sync.dma_start(out=sbt_sb, in_=sbt[:])
        ident_sb = persist.tile([P, P], in_dt, tag="ident", name="ident_sb")
        nc.sync.dma_start(out=ident_sb, in_=ident_d[:])

        # --- x chunk 0 prefetch (one DMA, before W on the sync ring) ---
        xch = [None] * NCH
        xch[0] = xp.tile([P, NK * TCH], in_dt, tag="xch", name="xch_0")
        nc.sync.dma_start(
            out=xch[0][:, :].rearrange("p (nk c) -> p nk c", nk=NK),
            in_=xt[:, 0:TCH].rearrange("(nk p) c -> p nk c", p=P),
        )

        # --- fold LoRA into effective weight: weff[k] = wt[k] + A^T_k @ sBt ---
        weff = []
        for wave in range(NWAVE):
            wv = wpool.tile([P, WAVE * D_OUT], in_dt, tag="wv", name=f"wv_{wave}")
            nc.sync.dma_start(
                out=wv[:, :].rearrange("p (nk c) -> p nk c", nk=WAVE),
                in_=wt[wave * WAVE * P:(wave + 1) * WAVE * P, :]
                .rearrange("(nk p) c -> p nk c", p=P),
            )
            for j in range(WAVE):
                k = wave * WAVE + j
                we = persist.tile([P, D_OUT], in_dt, tag=f"weff{k}",
                                  name=f"weff_{k}")
                pss = [
                    psum.tile([P, OC], f32, tag="ps", name=f"pps_{k}_{oc}")
                    for oc in range(NOC)
                ]
                atk = at_sb[:, k * P:(k + 1) * P]
                for oc in range(NOC):
                    nc.tensor.matmul(
                        pss[oc],
                        atk,
                        sbt_sb[:, oc * OC:(oc + 1) * OC],
                        start=True,
                        stop=(oc < 2),
                    )
                for oc in (2, 3):
                    nc.tensor.matmul(
                        pss[oc],
                        ident_sb[:],
                        wv[:, j * D_OUT + oc * OC:j * D_OUT + (oc + 1) * OC],
                        start=False,
                        stop=True,
                    )
                for oc in (0, 1):
                    nc.vector.tensor_add(
                        we[:, oc * OC:(oc + 1) * OC],
                        pss[oc],
                        wv[:, j * D_OUT + oc * OC:j * D_OUT + (oc + 1) * OC],
                    )
                for oc in (2, 3):
                    nc.scalar.activation(
                        we[:, oc * OC:(oc + 1) * OC], pss[oc], COPY,
                    )
                weff.append(we)

        # --- main GEMM over token chunks ---
        for t in range(NCH):
            if t > 0:
                xch[t] = xp.tile([P, NK * TCH], in_dt, tag="xch",
                                 name=f"xch_{t}")
                nc.sync.dma_start(
                    out=xch[t][:, :].rearrange("p (nk c) -> p nk c", nk=NK),
                    in_=xt[:, t * TCH:(t + 1) * TCH]
                    .rearrange("(nk p) c -> p nk c", p=P),
                )
            for m in range(M_PER):
                pss = [
                    psum.tile([P, OC], f32, tag="ps", name=f"ps_{t}_{m}_{oc}")
                    for oc in range(NOC)
                ]
                for k in range(NK):
                    lhsT = xch[t][:, k * TCH + m * P:k * TCH + (m + 1) * P]
                    for oc in range(NOC):
                        nc.tensor.matmul(
                            pss[oc],
                            lhsT,
                            weff[k][:, oc * OC:(oc + 1) * OC],
                            start=(k == 0),
                            stop=(k == NK - 1),
                        )
                row0 = (t * M_PER + m) * P
                ob = outp.tile([P, D_OUT], f32, tag="ob", name=f"ob_{t}_{m}")
                for oc in range(NOC):
                    nc.vector.tensor_add(
                        ob[:, oc * OC:(oc + 1) * OC], pss[oc],
                        bias_sb[:, oc * OC:(oc + 1) * OC]
                    )
                nc.scalar.dma_start(out=y[row0:row0 + P, :], in_=ob)

    _dedup_ldweights(nc)
    return nc


def _get_program():
    global _PROGRAM
    if _PROGRAM is None:
        _PROGRAM = _build_program()
        # run_bass_via_pjrt does not finalize; Bacc's compile passes
        # (register alloc, wait legalization) run here.
        _PROGRAM.finalize()
    return _PROGRAM


def kernel(x, W, bias, lora_a, lora_b, scalings, trace=False):
    global LAST_RESULTS
    from concourse.bass_utils import run_bass_kernel_spmd

    assert x.shape == (N_TOK, D_IN) and W.shape == (D_OUT, D_IN)
    bf16 = ml_dtypes.bfloat16

    # Host-side layout prep (not on the device critical path).
    xT = np.ascontiguousarray(x.astype(bf16).T)                    # [D_IN, N]
    wT = np.ascontiguousarray(W.astype(bf16).T)                    # [D_IN, D_OUT]
    at_all = lora_a.astype(bf16)                                   # [E, R, D_IN]
    sbt_all = np.ascontiguousarray(
        (lora_b.astype(np.float64) * scalings[:, None, None].astype(np.float64))
        .transpose(0, 2, 1)
    ).astype(bf16)                                                 # [E, R, D_OUT]
    bias32 = np.ascontiguousarray(bias.astype(np.float32))
    ident = np.eye(P, dtype=bf16)

    in_maps = []
    for e in range(E):
        in_maps.append(
            {
                "xt": np.ascontiguousarray(xT[:, e * S:(e + 1) * S]),
                "wt": wT,
                "bias": bias32,
                "at": np.ascontiguousarray(at_all[e]),
                "sbt": np.ascontiguousarray(sbt_all[e]),
                "ident": ident,
            }
        )

    nc = _get_program()
    res = run_bass_kernel_spmd(nc, in_maps, core_ids=list(range(E)), trace=trace)
    LAST_RESULTS = res
    out = np.concatenate([r["y"] for r in res.results], axis=0)
    return out.astype(np.float32)


# revision 5
# speedup vs baseline: 1.0231x; 1.0231x over previous
"""Trainium2 Bass kernel for LoRA-segmented linear layer.

Computes y = x @ W^T + bias + scalings[e] * (x_e @ A_e^T) @ B_e^T
where x is split into 8 equal contiguous token segments (one per adapter).

Sharding: data-parallel over tokens; core e gets segment e (4096 tokens),
which exactly matches adapter e, so all LoRA work is core-local.

Per-core device kernel:
  0. Warmup matmuls on a zeroed tile keep the PE busy from t=0 so the HAM
     clock gate reaches 8/8 (2.4 GHz) before real work arrives, and stays
     there through the DMA-bound startup phase.
  1. Fold LoRA into an effective weight on-device:
       W_eff^T = W^T + A_e^T @ (s_e * B_e^T)
     The rank-16 product goes to PSUM on the PE; evacuation to SBUF is
     split across two engines to keep it off the critical path:
       - d_out columns 0-1023: DVE tensor_add(psum, W_tile)
       - d_out columns 1024-2047: an identity matmul accumulates the W tile
         into PSUM on the (idle) PE, then the scalar/ACT engine copies
         psum -> sbuf (bf16 downcast). ACT cannot add tensors, but it can
         copy, and the PE has slack during the DMA-bound start.
  2. Dense GEMM y_e = x_e @ W_eff^T + bias, tiled as:
       stationary = x^T tile [128(d) x 128(tok)], moving = W_eff^T [128 x 512]
       PSUM accumulates fp32 over the 16 k-tiles; DVE adds bias; one
       1 MB DMA per 128-token row block writes y out on the scalar ring.

DMAs are batched (one per x chunk / W wave / output row-block) to cut
trigger overhead, with W and x on the sync engine's HWDGE ring ordered
x0 -> W -> x1.. so startup transfers have strict priority, and y on the
scalar engine's ring.

After TileContext exit we deduplicate InstLdweights: the tile legalizer
emits one LDWEIGHTS per matmul, but consecutive matmuls that share the
same stationary AP (the 4 d_out chunks per x-tile) only need the first.
This cuts the PE issue gap from ~259ns/MM to the ~216ns streaming floor.

Host-side prep: transpose x/W, cast to bf16, pre-scale B by scalings.
"""

import numpy as np
import ml_dtypes

# Problem geometry (hardcoded per contest contract).
N_TOK, D_IN, D_OUT, E, R = 32768, 2048, 2048, 8, 16
S = N_TOK // E          # tokens per core / segment: 4096
P = 128                 # partitions
NK = D_IN // P          # 16 contraction tiles
TCH = 512               # token chunk (x dma width per k-tile)
NCH = S // TCH          # 8 token chunks per core
M_PER = TCH // P        # 4 m-subtiles (of 128 tokens) per chunk
OC = 512                # dout chunk (matmul moving free dim; one PSUM bank)
NOC = D_OUT // OC       # 4 dout chunks
WAVE = 2                # k-tiles per W DMA wave
NWAVE = NK // WAVE      # 8 waves
N_WARM = 32             # warmup matmuls bridge t=0 to the first W arrival

_PROGRAM = None         # cached Bass program
LAST_RESULTS = None     # BassKernelResults of the most recent run (for profiling)


def _dedup_ldweights(nc):
    """Remove InstLdweights that reload the stationary AP already resident
    (identical AP string, only matmults in between). Any waits on a removed
    LDW are moved onto the next matmult. Must run after TileContext exit and
    before nc.finalize()."""
    import concourse.mybir as mybir

    n_removed = 0
    for fn in nc.m.functions:
        for bb in fn.blocks:
            insts = list(bb.instructions)
            keep = []
            last_key = None
            pending_waits = []
            removed_here = False
            for i in insts:
                tn = type(i).__name__
                if tn == "InstLdweights":
                    key = str(i.ins[0])
                    if key == last_key:
                        si = i.sync_info
                        if si is not None and si.on_wait:
                            pending_waits.extend(si.on_wait)
                        n_removed += 1
                        removed_here = True
                        continue
                    last_key = key
                elif tn == "InstMatmult":
                    if pending_waits:
                        si = i.sync_info
                        if si is None:
                            i.sync_info = mybir.SyncInfo(
                                on_wait=list(pending_waits), on_update=[])
                        else:
                            si.on_wait = list(si.on_wait) + pending_waits
                            i.sync_info = si
                        pending_waits = []
                else:
                    last_key = None
                keep.append(i)
            assert not pending_waits
            if removed_here:
                bb.instructions = keep
    return n_removed


def _build_program(in_dt_name="bfloat16"):
    from contextlib import ExitStack

    import concourse.mybir as mybir
    import concourse.tile as tile
    from concourse import bacc

    in_dt = getattr(mybir.dt, in_dt_name)
    f32 = mybir.dt.float32
    COPY = mybir.ActivationFunctionType.Copy

    nc = bacc.Bacc(trn_type="TRN2")

    xt = nc.dram_tensor("xt", [D_IN, S], in_dt, kind="ExternalInput")
    wt = nc.dram_tensor("wt", [D_IN, D_OUT], in_dt, kind="ExternalInput")
    bias_d = nc.dram_tensor("bias", [D_OUT], f32, kind="ExternalInput")
    at = nc.dram_tensor("at", [R, D_IN], in_dt, kind="ExternalInput")
    sbt = nc.dram_tensor("sbt", [R, D_OUT], in_dt, kind="ExternalInput")
    ident_d = nc.dram_tensor("ident", [P, P], in_dt, kind="ExternalInput")
    y = nc.dram_tensor("y", [S, D_OUT], f32, kind="ExternalOutput")

    with ExitStack() as ctx:
        tc = ctx.enter_context(tile.TileContext(nc))
        persist = ctx.enter_context(tc.tile_pool(name="persist", bufs=1))
        wpool = ctx.enter_context(tc.tile_pool(name="wpool", bufs=2))
        xp = ctx.enter_context(tc.tile_pool(name="xp", bufs=2))
        xp0 = ctx.enter_context(tc.tile_pool(name="xp0", bufs=4))
        outp = ctx.enter_context(tc.tile_pool(name="outp", bufs=3))
        psum = ctx.enter_context(tc.tile_pool(name="psum", bufs=8, space="PSUM"))

        # --- HAM warmup: PE busy from t=0 so the clock ungates early ---
        warm = persist.tile([P, OC], in_dt, tag="warm", name="warm_sb")
        nc.vector.memset(warm, 0.0)
        wps = psum.tile([P, OC], f32, tag="ps", name="warm_ps")
        for i in range(N_WARM):
            nc.tensor.matmul(wps, warm[:, :P], warm[:], start=True, stop=True)

        # --- persistent small tensors ---
        bias_sb = persist.tile([P, D_OUT], f32, tag="bias", name="bias_sb")
        # stride-0 partition broadcast must go via SW DGE (gpsimd), not HW DGE
        nc.gpsimd.dma_start(out=bias_sb, in_=bias_d[:].partition_broadcast(P))
        at_sb = persist.tile([R, D_IN], in_dt, tag="at", name="at_sb")
        nc.sync.dma_start(out=at_sb, in_=at[:])
        sbt_sb = persist.tile([R, D_OUT], in_dt, tag="sbt", name="sbt_sb")
        nc.sync.dma_start(out=sbt_sb, in_=sbt[:])
        ident_sb = persist.tile([P, P], in_dt, tag="ident", name="ident_sb")
        nc.sync.dma_start(out=ident_sb, in_=ident_d[:])

        # --- fold LoRA into effective weight: weff[k] = wt[k] + A^T_k @ sBt ---
        weff = []
        for wave in range(NWAVE):
            wv = wpool.tile([P, WAVE * D_OUT], in_dt, tag="wv", name=f"wv_{wave}")
            nc.sync.dma_start(
                out=wv[:, :].rearrange("p (nk c) -> p nk c", nk=WAVE),
                in_=wt[wave * WAVE * P:(wave + 1) * WAVE * P, :]
                .rearrange("(nk p) c -> p nk c", p=P),
            )
            for j in range(WAVE):
                k = wave * WAVE + j
                we = persist.tile([P, D_OUT], in_dt, tag=f"weff{k}",
                                  name=f"weff_{k}")
                pss = [
                    psum.tile([P, OC], f32, tag="ps", name=f"pps_{k}_{oc}")
                    for oc in range(NOC)
                ]
                atk = at_sb[:, k * P:(k + 1) * P]
                for oc in range(NOC):
                    nc.tensor.matmul(
                        pss[oc],
                        atk,
                        sbt_sb[:, oc * OC:(oc + 1) * OC],
                        start=True,
                        stop=(oc < 2),
                    )
                for oc in (2, 3):
                    nc.tensor.matmul(
                        pss[oc],
                        ident_sb[:],
                        wv[:, j * D_OUT + oc * OC:j * D_OUT + (oc + 1) * OC],
                        start=False,
                        stop=True,
                    )
                for oc in (0, 1):
                    nc.vector.tensor_add(
                        we[:, oc * OC:(oc + 1) * OC],
                        pss[oc],
                        wv[:, j * D_OUT + oc * OC:j * D_OUT + (oc + 1) * OC],
                    )
                for oc in (2, 3):
                    nc.scalar.activation(
                        we[:, oc * OC:(oc + 1) * OC], pss[oc], COPY,
                    )
                weff.append(we)

        # --- x chunk 0: four per-m-subtile DMAs (after W on the sync ring,
        # so the W load has strict priority; quarter-granularity so the
        # first 128-token group closes as early as possible) ---
        x0m = []
        for m in range(M_PER):
            xm = xp0.tile([P, NK * P], in_dt, tag="x0m", name=f"x0m_{m}")
            nc.sync.dma_start(
                out=xm[:, :].rearrange("p (nk c) -> p nk c", nk=NK),
                in_=xt[:, m * P:(m + 1) * P]
                .rearrange("(nk p) c -> p nk c", p=P),
            )
            x0m.append(xm)

        # --- main GEMM over token chunks ---
        xch = [None] * NCH
        for t in range(NCH):
            if t > 0:
                xch[t] = xp.tile([P, NK * TCH], in_dt, tag="xch",
                                 name=f"xch_{t}")
                nc.sync.dma_start(
                    out=xch[t][:, :].rearrange("p (nk c) -> p nk c", nk=NK),
                    in_=xt[:, t * TCH:(t + 1) * TCH]
                    .rearrange("(nk p) c -> p nk c", p=P),
                )
            for m in range(M_PER):
                pss = [
                    psum.tile([P, OC], f32, tag="ps", name=f"ps_{t}_{m}_{oc}")
                    for oc in range(NOC)
                ]
                for k in range(NK):
                    if t == 0:
                        lhsT = x0m[m][:, k * P:(k + 1) * P]
                    else:
                        lhsT = xch[t][:, k * TCH + m * P:k * TCH + (m + 1) * P]
                    for oc in range(NOC):
                        nc.tensor.matmul(
                            pss[oc],
                            lhsT,
                            weff[k][:, oc * OC:(oc + 1) * OC],
                            start=(k == 0),
                            stop=(k == NK - 1),
                        )
                row0 = (t * M_PER + m) * P
                ob = outp.tile([P, D_OUT], f32, tag="ob", name=f"ob_{t}_{m}")
                last = (t == NCH - 1 and m == M_PER - 1)
                for oc in range(NOC):
                    nc.vector.tensor_add(
                        ob[:, oc * OC:(oc + 1) * OC], pss[oc],
                        bias_sb[:, oc * OC:(oc + 1) * OC]
                    )
                    if last:
                        nc.scalar.dma_start(
                            out=y[row0:row0 + P, oc * OC:(oc + 1) * OC],
                            in_=ob[:, oc * OC:(oc + 1) * OC],
                        )
                if not last:
                    nc.scalar.dma_start(out=y[row0:row0 + P, :], in_=ob)

    _dedup_ldweights(nc)
    return nc


def _get_program():
    global _PROGRAM
    if _PROGRAM is None:
        _PROGRAM = _build_program()
        # run_bass_via_pjrt does not finalize; Bacc's compile passes
        # (register alloc, wait legalization) run here.
        _PROGRAM.finalize()
    return _PROGRAM


def kernel(x, W, bias, lora_a, lora_b, scalings, trace=False):
    global LAST_RESULTS
    from concourse.bass_utils import run_bass_kernel_spmd

    assert x.shape == (N_TOK, D_IN) and W.shape == (D_OUT, D_IN)
    bf16 = ml_dtypes.bfloat16

    # Host-side layout prep (not on the device critical path).
    xT = np.ascontiguousarray(x.astype(bf16).T)                    # [D_IN, N]
    wT = np.ascontiguousarray(W.astype(bf16).T)                    # [D_IN, D_OUT]
    at_all = lora_a.astype(bf16)                                   # [E, R, D_IN]
    sbt_all = np.ascontiguousarray(
        (lora_b.astype(np.float64) * scalings[:, None, None].astype(np.float64))
        .transpose(0, 2, 1)
    ).astype(bf16)                                                 # [E, R, D_OUT]
    bias32 = np.ascontiguousarray(bias.astype(np.float32))
    ident = np.eye(P, dtype=bf16)

    in_maps = []
    for e in range(E):
        in_maps.append(
            {
                "xt": np.ascontiguousarray(xT[:, e * S:(e + 1) * S]),
                "wt": wT,
                "bias": bias32,
                "at": np.ascontiguousarray(at_all[e]),
                "sbt": np.ascontiguousarray(sbt_all[e]),
                "ident": ident,
            }
        )

    nc = _get_program()
    res = run_bass_kernel_spmd(nc, in_maps, core_ids=list(range(E)), trace=trace)
    LAST_RESULTS = res
    out = np.concatenate([r["y"] for r in res.results], axis=0)
    return out.astype(np.float32)
